# revision 6
# baseline (speedup 1.0000x reference)
"""BENDR contrastive-loss kernel for Trainium2 (8 NeuronCores).

Reference computation (see problem): for each (b, t):
  logits[b*T+t, 0]   = cos(z[b,:,t], c[b,:,t+1]) / TEMP
  logits[b*T+t, 1+k] = cos(z[b,:,t], z[b,:,n(b,t,k)]) / TEMP
with n(b,t,k) = negative_inds[b, t*K+k] (row-local), TEMP=0.5.

Strategy: data-parallel over batch (2 rows per core). On device, all the
arithmetic runs on the TensorEngine as block similarity matrices:
  - rn_z[t] = 1/||z[:,t]||, rc[t] = 1/||c[:,t+1]|| via squared tiles +
    ones-matmul partition reduction, reciprocal (DVE) + sqrt (ACT).
  - zs[:,t] = z[:,t] * rn_z[t] * sqrt(2);  cs[:,t] = c[:,t+1] * rc[t] * sqrt(2)
    (folds both cosine denominators and the 1/TEMP=2 factor).
  - per 128-wide t-block: sims = zs_blockT @ [zs_all | cs_block]  ->
    [128, 2048+128] fp32 PSUM, stored to DRAM as fp16.
Every output logit is exactly one entry of sims: the negative (t,k) is
sims[t, n(t,k)] and the positive is sims[t, 2048+t%128].  The host does the
final index-pick (pure indexing / unshard) and returns [B*T, K+1] float32.

The gather could not be done on-device at speed: GPSIMD indirect_copy
measures ~29us per 1024 indices (~2.4ms total here), ap_gather does not
compile on this toolchain, and indirect DMA gathers measured ~62ns/row with
8 SW queues.  Computing the full similarity block on the PE (128x128 MACs
per cycle) and shipping it out in fp16 is ~50x cheaper than any of those.
"""

import sys

for _p in ("/opt/trn_rl_repo",):
    if _p not in sys.path:
        sys.path.append(_p)

import numpy as np

import concourse.bass as bass
import concourse.mybir as mybir
from concourse import tile as _tile
from concourse.tile import TileContext
from concourse.bass_utils import run_bass_kernel_spmd

dt = mybir.dt

B, F, T, K = 16, 256, 2048, 20
NCORES = 8
ROWS = B // NCORES          # batch rows per core
NBLK = T // 128             # t-blocks per batch row
WC = T + 128                # sims columns: 2048 z-sims + 128 c-diag block
FCH = F // 128              # f chunks (partition dim)

# ---------------------------------------------------------------------------
# Walrus in this container rejects instructions that carry more than one
# semaphore wait ("Too many sync wait commands").  Two shims fix that: the
# tile tail drain gets its waits on single-wait NOPs, and a post-pass splits
# any remaining multi-wait instruction.
# ---------------------------------------------------------------------------


def _patched_drain_and_barrier(self, tick_clock, wait_clock):
    nop0 = self.nc.sync.nop(nofuse=True, hint="tail_wait")
    wait_clock.add_sem_waits(
        nop0.ins, _tile.ScopedClock({None: tick_clock.global_clock})
    )
    si = nop0.ins.sync_info
    if si is not None and len(si.on_wait) > 1:
        waits = list(si.on_wait)
        nop0.ins.sync_info = mybir.SyncInfo(
            on_wait=waits[:1], on_update=list(si.on_update)
        )
        for w in waits[1:]:
            nopi = self.nc.sync.nop(nofuse=True, hint="tail_wait")
            nopi.ins.sync_info = mybir.SyncInfo(on_wait=[w], on_update=[])
    self.nc.sync.drain()
    self.nc.all_engine_barrier()
    assert self.sems is not None
    popped = self.nc._tile_sem_poison_stack.pop()
    assert popped is self._sem_poison
    self.nc.clear_and_free_semaphores(list(self.sems.allocated().values()))
    self.nc.all_engine_barrier()


_tile.TileContext._drain_and_barrier = _patched_drain_and_barrier

_wnop_counter = [0]


def split_excess_waits(nc, cap=1):
    for f in nc.m.functions:
        for bb in f.blocks:
            insts = bb.instructions
            out = []
            changed = False
            for inst in list(insts):
                si = getattr(inst, "sync_info", None)
                waits = list(si.on_wait) if si is not None else []
                if len(waits) > cap:
                    keep = waits[-cap:]
                    for w in waits[: len(waits) - cap]:
                        _wnop_counter[0] += 1
                        nop = mybir.InstNoOp(
                            name=f"wnop-{_wnop_counter[0]}", ins=[], outs=[]
                        )
                        nop.engine = inst.engine
                        nop.sync_info = mybir.SyncInfo(on_wait=[w], on_update=[])
                        out.append(nop)
                    inst.sync_info = mybir.SyncInfo(
                        on_wait=keep, on_update=list(si.on_update)
                    )
                    changed = True
                out.append(inst)
            if changed:
                insts[:] = out


# ---------------------------------------------------------------------------
# Device program
# ---------------------------------------------------------------------------


def build_program():
    nc = bass.Bass("TRN2", num_devices=NCORES)
    z_in = nc.dram_tensor("z", [ROWS, F, T], dt.float32, kind="ExternalInput")
    c_in = nc.dram_tensor("c", [ROWS, F, T], dt.float32, kind="ExternalInput")
    sims_out = nc.dram_tensor(
        "sims", [ROWS * NBLK * 128, WC], dt.float16, kind="ExternalOutput"
    )

    with TileContext(nc) as tc:
        with (
            tc.tile_pool(name="io", bufs=1) as io_pool,
            tc.tile_pool(name="work", bufs=1) as work,
            tc.tile_pool(name="scaled", bufs=2) as scaled,
            tc.tile_pool(name="outp", bufs=3) as outp,
            tc.tile_pool(name="gram_ps", bufs=2, space="PSUM") as gram_ps,
            tc.tile_pool(name="stat_ps", bufs=1, space="PSUM") as stat_ps,
        ):
            ones16 = io_pool.tile([128, 1], dt.bfloat16, name="ones16")
            nc.vector.memset(ones16[:], 1.0)

            for r in range(ROWS):
                # ---- load + convert ----
                zf = []
                cf = []
                z16 = []
                c16 = []
                for j in range(FCH):
                    zfj = io_pool.tile([128, T], dt.float32, name=f"zf{j}", tag=f"zf{j}")
                    nc.sync.dma_start(out=zfj[:], in_=z_in[r, 128 * j : 128 * (j + 1), :])
                    zf.append(zfj)
                    cfj = io_pool.tile([128, T], dt.float32, name=f"cf{j}", tag=f"cf{j}")
                    nc.sync.dma_start(out=cfj[:], in_=c_in[r, 128 * j : 128 * (j + 1), :])
                    cf.append(cfj)
                    z16j = work.tile([128, T], dt.bfloat16, name=f"z16{j}", tag=f"z16{j}")
                    nc.scalar.copy(z16j[:], zfj[:])
                    z16.append(z16j)
                    c16j = work.tile([128, T], dt.bfloat16, name=f"c16{j}", tag=f"c16{j}")
                    nc.scalar.copy(c16j[:], cfj[:])
                    c16.append(c16j)

                # ---- squared tiles (bf16, in place) + ones-matmul reduce ----
                for j in range(FCH):
                    nc.vector.tensor_tensor(
                        out=z16[j][:], in0=z16[j][:], in1=z16[j][:], op=mybir.AluOpType.mult
                    )
                    nc.vector.tensor_tensor(
                        out=c16[j][:], in0=c16[j][:], in1=c16[j][:], op=mybir.AluOpType.mult
                    )
                # rn = sqrt(2 / normsq), chunked through [1, 512] PSUM tiles
                rz32 = work.tile([1, T], dt.float32, name="rz32", tag="rz32")
                rc32 = work.tile([1, T], dt.float32, name="rc32", tag="rc32")
                for cchunk in range(T // 512):
                    sl = slice(512 * cchunk, 512 * (cchunk + 1))
                    nz_ps = stat_ps.tile([1, 512], dt.float32, name="nz_ps", tag="nz_ps")
                    ncc_ps = stat_ps.tile([1, 512], dt.float32, name="ncc_ps", tag="ncc_ps")
                    for j in range(FCH):
                        nc.tensor.matmul(
                            nz_ps[:], ones16[:], z16[j][:, sl],
                            start=(j == 0), stop=(j == FCH - 1),
                        )
                        nc.tensor.matmul(
                            ncc_ps[:], ones16[:], c16[j][:, sl],
                            start=(j == 0), stop=(j == FCH - 1),
                        )
                    nc.vector.reciprocal(rz32[:, sl], nz_ps[:])
                    nc.vector.reciprocal(rc32[:, sl], ncc_ps[:])
                rnz = work.tile([128, T], dt.float32, name="rnz", tag="rnz")
                nc.scalar.activation(
                    rnz[0:1, :], rz32[:], mybir.ActivationFunctionType.Sqrt, scale=2.0
                )
                rnc = work.tile([128, T], dt.float32, name="rnc", tag="rnc")
                nc.scalar.activation(
                    rnc[0:1, :], rc32[:], mybir.ActivationFunctionType.Sqrt, scale=2.0
                )
                # replicate row 0 -> all 128 partitions (log doubling)
                kk = 1
                while kk < 128:
                    nc.sync.dma_start(out=rnz[kk : 2 * kk, :], in_=rnz[0:kk, :])
                    nc.sync.dma_start(out=rnc[kk : 2 * kk, :], in_=rnc[0:kk, :])
                    kk *= 2

                # ---- scaled operands (scale in place into zf/cf) ----
                zs16 = []
                cs16 = []
                for j in range(FCH):
                    nc.vector.tensor_tensor(
                        out=zf[j][:], in0=zf[j][:], in1=rnz[:], op=mybir.AluOpType.mult
                    )
                    zs16j = scaled.tile([128, T], dt.bfloat16, name=f"zs16{j}", tag=f"zs16{j}")
                    nc.scalar.copy(zs16j[:], zf[j][:])
                    zs16.append(zs16j)
                    nc.vector.tensor_tensor(
                        out=cf[j][:], in0=cf[j][:], in1=rnc[:], op=mybir.AluOpType.mult
                    )
                    cs16j = scaled.tile([128, T], dt.bfloat16, name=f"cs16{j}", tag=f"cs16{j}")
                    nc.scalar.copy(cs16j[:], cf[j][:])
                    cs16.append(cs16j)

                # ---- per t-block similarity matrices ----
                for tau in range(NBLK):
                    t0 = 128 * tau
                    otile = outp.tile([128, WC], dt.float16, name="otile", tag="otile")
                    for h in range(2):
                        ps = gram_ps.tile([128, 1088], dt.float32, name="ps", tag="ps")
                        for j in range(FCH):
                            lhsT = zs16[j][:, t0 : t0 + 128]
                            st = j == 0
                            sp = j == FCH - 1
                            nc.tensor.matmul(
                                ps[:, 0:512], lhsT,
                                zs16[j][:, 1024 * h : 1024 * h + 512],
                                start=st, stop=sp,
                            )
                            nc.tensor.matmul(
                                ps[:, 512:1024], lhsT,
                                zs16[j][:, 1024 * h + 512 : 1024 * h + 1024],
                                start=st, stop=sp,
                            )
                            nc.tensor.matmul(
                                ps[:, 1024:1088], lhsT,
                                cs16[j][:, t0 + 64 * h : t0 + 64 * h + 64],
                                start=st, stop=sp,
                            )
                        nc.scalar.copy(otile[:, 1024 * h : 1024 * (h + 1)], ps[:, 0:1024])
                        nc.scalar.copy(
                            otile[:, 2048 + 64 * h : 2048 + 64 * (h + 1)],
                            ps[:, 1024:1088],
                        )
                    nc.sync.dma_start(
                        out=sims_out[(r * NBLK + tau) * 128 : (r * NBLK + tau + 1) * 128, :],
                        in_=otile[:],
                    )

    split_excess_waits(nc)
    return nc


_PROGRAM = None


def _get_program():
    global _PROGRAM
    if _PROGRAM is None:
        _PROGRAM = build_program()
    return _PROGRAM


def kernel(z, c, negative_inds, _trace=False):
    z = np.ascontiguousarray(np.asarray(z, dtype=np.float32))
    c = np.ascontiguousarray(np.asarray(c, dtype=np.float32))
    ni = np.asarray(negative_inds)
    assert z.shape == (B, F, T) and c.shape == (B, F, T + 1)

    c_sl = np.ascontiguousarray(c[:, :, 1:])  # [B, F, T]

    nc = _get_program()
    in_maps = []
    for core in range(NCORES):
        rs = slice(core * ROWS, (core + 1) * ROWS)
        in_maps.append({"z": z[rs], "c": c_sl[rs]})

    res = run_bass_kernel_spmd(nc, in_maps, list(range(NCORES)), trace=_trace)

    # [B, T, WC] fp16: all candidate similarities (already scaled by
    # 2 / (||z_t|| ||target||), i.e. final logits)
    sims = np.concatenate(
        [res.results[i]["sims"].reshape(ROWS, T, WC) for i in range(NCORES)], axis=0
    )

    # host-side index pick (pure unshard / indexing)
    n = ni.reshape(B, T, K).astype(np.int64)  # values in [0, T-2]
    neg = np.take_along_axis(sims[:, :, :T], n, axis=2)  # [B, T, K]
    tmod = (np.arange(T) % 128)[None, :, None]
    pos = np.take_along_axis(sims[:, :, T:], tmod, axis=2)  # [B, T, 1]
    logits = np.concatenate([pos, neg], axis=2).astype(np.float32)
    out = logits.reshape(B * T, K + 1)
    if _trace:
        return out, res
    return out


if __name__ == "__main__":
    rng = np.random.default_rng(0)
    z = rng.standard_normal((B, F, T), dtype=np.float32)
    c = rng.standard_normal((B, F, T + 1), dtype=np.float32)
    ni = rng.integers(0, T - 1, size=(B, T * K)).astype(np.int64)
    out = kernel(z=z, c=c, negative_inds=ni)
    print("out", out.shape, out.dtype, np.isfinite(out).all())


# revision 11
# speedup vs baseline: 1.4236x; 1.4236x over previous
"""BENDR contrastive-loss kernel for Trainium2 (8 NeuronCores).

Reference computation (see problem): for each (b, t):
  logits[b*T+t, 0]   = cos(z[b,:,t], c[b,:,t+1]) / TEMP
  logits[b*T+t, 1+k] = cos(z[b,:,t], z[b,:,n(b,t,k)]) / TEMP
with n(b,t,k) = negative_inds[b, t*K+k] (row-local), TEMP=0.5.

Strategy: data-parallel over batch (2 rows per core). On device, all the
arithmetic runs on the TensorEngine as block similarity matrices:
  - rn_z[t] = 1/||z[:,t]||, rc[t] = 1/||c[:,t+1]|| via squared tiles +
    ones-matmul partition reduction, reciprocal (DVE) + sqrt (ACT).
  - zs[:,t] = z[:,t] * rn_z[t] * sqrt(2);  cs[:,t] = c[:,t+1] * rc[t] * sqrt(2)
    (folds both cosine denominators and the 1/TEMP=2 factor).
  - per 128-wide t-block: sims = zs_blockT @ [zs_all | cs_block]  ->
    [128, 2048+128] fp32 PSUM, stored to DRAM as fp16.
Every output logit is exactly one entry of sims: the negative (t,k) is
sims[t, n(t,k)] and the positive is sims[t, 2048+t%128].  The host does the
final index-pick (pure indexing / unshard) and returns [B*T, K+1] float32.

The gather could not be done on-device at speed: GPSIMD indirect_copy
measures ~29us per 1024 indices (~2.4ms total here), ap_gather does not
compile on this toolchain, and indirect DMA gathers measured ~62ns/row with
8 SW queues.  Computing the full similarity block on the PE (128x128 MACs
per cycle) and shipping it out in fp16 is ~50x cheaper than any of those.
"""

import sys

for _p in ("/opt/trn_rl_repo",):
    if _p not in sys.path:
        sys.path.append(_p)

import numpy as np

import concourse.bass as bass
import concourse.mybir as mybir
from concourse import tile as _tile
from concourse.tile import TileContext
from concourse.bass_utils import run_bass_kernel_spmd

dt = mybir.dt

B, F, T, K = 16, 256, 2048, 20
NCORES = 8
ROWS = B // NCORES          # batch rows per core
NBLK = T // 128             # t-blocks per batch row
WC = T + 128                # sims columns: 2048 z-sims + 128 c-diag block
FCH = F // 128              # f chunks (partition dim)

# ---------------------------------------------------------------------------
# Walrus in this container rejects instructions that carry more than one
# semaphore wait ("Too many sync wait commands").  Two shims fix that: the
# tile tail drain gets its waits on single-wait NOPs, and a post-pass splits
# any remaining multi-wait instruction.
# ---------------------------------------------------------------------------


def _patched_drain_and_barrier(self, tick_clock, wait_clock):
    nop0 = self.nc.sync.nop(nofuse=True, hint="tail_wait")
    wait_clock.add_sem_waits(
        nop0.ins, _tile.ScopedClock({None: tick_clock.global_clock})
    )
    si = nop0.ins.sync_info
    if si is not None and len(si.on_wait) > 1:
        waits = list(si.on_wait)
        nop0.ins.sync_info = mybir.SyncInfo(
            on_wait=waits[:1], on_update=list(si.on_update)
        )
        for w in waits[1:]:
            nopi = self.nc.sync.nop(nofuse=True, hint="tail_wait")
            nopi.ins.sync_info = mybir.SyncInfo(on_wait=[w], on_update=[])
    self.nc.sync.drain()
    self.nc.all_engine_barrier()
    assert self.sems is not None
    popped = self.nc._tile_sem_poison_stack.pop()
    assert popped is self._sem_poison
    self.nc.clear_and_free_semaphores(list(self.sems.allocated().values()))
    self.nc.all_engine_barrier()


_tile.TileContext._drain_and_barrier = _patched_drain_and_barrier

_wnop_counter = [0]


def split_excess_waits(nc, cap=1):
    for f in nc.m.functions:
        for bb in f.blocks:
            insts = bb.instructions
            out = []
            changed = False
            for inst in list(insts):
                si = getattr(inst, "sync_info", None)
                waits = list(si.on_wait) if si is not None else []
                if len(waits) > cap:
                    keep = waits[-cap:]
                    for w in waits[: len(waits) - cap]:
                        _wnop_counter[0] += 1
                        nop = mybir.InstNoOp(
                            name=f"wnop-{_wnop_counter[0]}", ins=[], outs=[]
                        )
                        nop.engine = inst.engine
                        nop.sync_info = mybir.SyncInfo(on_wait=[w], on_update=[])
                        out.append(nop)
                    inst.sync_info = mybir.SyncInfo(
                        on_wait=keep, on_update=list(si.on_update)
                    )
                    changed = True
                out.append(inst)
            if changed:
                insts[:] = out


# ---------------------------------------------------------------------------
# Device program
# ---------------------------------------------------------------------------


def build_program():
    nc = bass.Bass("TRN2", num_devices=NCORES)
    z_in = nc.dram_tensor("z", [ROWS, F, T], dt.float32, kind="ExternalInput")
    c_in = nc.dram_tensor("c", [ROWS, F, T], dt.float32, kind="ExternalInput")
    sims_out = nc.dram_tensor(
        "sims", [ROWS * NBLK * 128, WC], dt.float16, kind="ExternalOutput"
    )

    with TileContext(nc) as tc:
        with (
            tc.tile_pool(name="io", bufs=1) as io_pool,
            tc.tile_pool(name="work", bufs=1) as work,
            tc.tile_pool(name="scaled", bufs=2) as scaled,
            tc.tile_pool(name="outp", bufs=3) as outp,
            tc.tile_pool(name="gram_ps", bufs=3, space="PSUM") as gram_ps,
            tc.tile_pool(name="stat_ps", bufs=1, space="PSUM") as stat_ps,
        ):
            ones16 = io_pool.tile([128, 1], dt.bfloat16, name="ones16")
            nc.vector.memset(ones16[:], 1.0)

            for r in range(ROWS):
                # ---- load + convert ----
                zf = []
                cf = []
                z16 = []
                c16 = []
                for j in range(FCH):
                    zfj = io_pool.tile([128, T], dt.float32, name=f"zf{j}", tag=f"zf{j}")
                    nc.sync.dma_start(out=zfj[:], in_=z_in[r, 128 * j : 128 * (j + 1), :])
                    zf.append(zfj)
                    cfj = io_pool.tile([128, T], dt.float32, name=f"cf{j}", tag=f"cf{j}")
                    nc.sync.dma_start(out=cfj[:], in_=c_in[r, 128 * j : 128 * (j + 1), :])
                    cf.append(cfj)
                    z16j = work.tile([128, T], dt.bfloat16, name=f"z16{j}", tag=f"z16{j}")
                    nc.scalar.copy(z16j[:], zfj[:])
                    z16.append(z16j)
                    c16j = work.tile([128, T], dt.bfloat16, name=f"c16{j}", tag=f"c16{j}")
                    nc.scalar.copy(c16j[:], cfj[:])
                    c16.append(c16j)

                # ---- squared tiles (bf16, in place) + ones-matmul reduce ----
                for j in range(FCH):
                    nc.vector.tensor_tensor(
                        out=z16[j][:], in0=z16[j][:], in1=z16[j][:], op=mybir.AluOpType.mult
                    )
                    nc.vector.tensor_tensor(
                        out=c16[j][:], in0=c16[j][:], in1=c16[j][:], op=mybir.AluOpType.mult
                    )
                # rn = sqrt(2 / normsq): ones-matmuls land the 4 column chunks
                # on partitions {0,32,64,96} so reciprocal runs on 4 DVE lanes
                nz_ps = stat_ps.tile([128, 512], dt.float32, name="nz_ps", tag="aux")
                ncc_ps = stat_ps.tile([128, 512], dt.float32, name="ncc_ps", tag="aux2")
                for cchunk in range(T // 512):
                    sl = slice(512 * cchunk, 512 * (cchunk + 1))
                    bp = 32 * cchunk
                    tp = (0, bp)
                    for j in range(FCH):
                        nc.tensor.matmul(
                            nz_ps[bp : bp + 1, :], ones16[:], z16[j][:, sl],
                            start=(j == 0), stop=(j == FCH - 1), tile_position=tp,
                        )
                        nc.tensor.matmul(
                            ncc_ps[bp : bp + 1, :], ones16[:], c16[j][:, sl],
                            start=(j == 0), stop=(j == FCH - 1), tile_position=tp,
                        )
                rz32 = work.tile([128, 512], dt.float32, name="rz32", tag="rz32")
                rc32 = work.tile([128, 512], dt.float32, name="rc32", tag="rc32")
                nc.vector.reciprocal(rz32[:], nz_ps[:])
                nc.vector.reciprocal(rc32[:], ncc_ps[:])
                rnz = work.tile([128, T], dt.float32, name="rnz", tag="rnz")
                rnc = work.tile([128, T], dt.float32, name="rnc", tag="rnc")
                for cchunk in range(T // 512):
                    sl = slice(512 * cchunk, 512 * (cchunk + 1))
                    bp = 32 * cchunk
                    nc.scalar.activation(
                        rnz[0:1, sl], rz32[bp : bp + 1, :],
                        mybir.ActivationFunctionType.Sqrt, scale=2.0,
                    )
                    nc.scalar.activation(
                        rnc[0:1, sl], rc32[bp : bp + 1, :],
                        mybir.ActivationFunctionType.Sqrt, scale=2.0,
                    )
                # replicate row 0 -> all 128 partitions (log doubling)
                kk = 1
                while kk < 128:
                    nc.sync.dma_start(out=rnz[kk : 2 * kk, :], in_=rnz[0:kk, :])
                    nc.sync.dma_start(out=rnc[kk : 2 * kk, :], in_=rnc[0:kk, :])
                    kk *= 2

                # ---- scaled operands (scale in place into zf/cf) ----
                zs16 = []
                cs16 = []
                for j in range(FCH):
                    nc.vector.tensor_tensor(
                        out=zf[j][:], in0=zf[j][:], in1=rnz[:], op=mybir.AluOpType.mult
                    )
                    zs16j = scaled.tile([128, T], dt.bfloat16, name=f"zs16{j}", tag=f"zs16{j}")
                    nc.scalar.copy(zs16j[:], zf[j][:])
                    zs16.append(zs16j)
                    nc.vector.tensor_tensor(
                        out=cf[j][:], in0=cf[j][:], in1=rnc[:], op=mybir.AluOpType.mult
                    )
                    cs16j = scaled.tile([128, T], dt.bfloat16, name=f"cs16{j}", tag=f"cs16{j}")
                    nc.scalar.copy(cs16j[:], cf[j][:])
                    cs16.append(cs16j)

                # ---- per t-block similarity matrices ----
                # j-outer so the stationary operand is loaded once per f-chunk
                for tau in range(NBLK):
                    t0 = 128 * tau
                    otile = outp.tile([128, WC], dt.float16, name="otile", tag="otile")
                    ps0 = gram_ps.tile([128, 1024], dt.float32, name="ps0", tag="ps_z")
                    ps1 = gram_ps.tile([128, 1024], dt.float32, name="ps1", tag="ps_z")
                    csim0 = stat_ps.tile([128, 64], dt.float32, name="csim0", tag="aux")
                    csim1 = stat_ps.tile([128, 64], dt.float32, name="csim1", tag="aux2")
                    csims = (csim0, csim1)
                    pss = (ps0, ps1)
                    for j in range(FCH):
                        lhsT = zs16[j][:, t0 : t0 + 128]
                        st = j == 0
                        sp = j == FCH - 1
                        for h in range(2):
                            ps = pss[h]
                            nc.tensor.matmul(
                                ps[:, 0:512], lhsT,
                                zs16[j][:, 1024 * h : 1024 * h + 512],
                                start=st, stop=sp,
                            )
                            nc.tensor.matmul(
                                ps[:, 512:1024], lhsT,
                                zs16[j][:, 1024 * h + 512 : 1024 * h + 1024],
                                start=st, stop=sp,
                            )
                            nc.tensor.matmul(
                                csims[h][:], lhsT,
                                cs16[j][:, t0 + 64 * h : t0 + 64 * h + 64],
                                start=st, stop=sp,
                            )
                    for h in range(2):
                        # alternate PSUM->SBUF copies between ACT and DVE
                        if (tau + h) % 2 == 0:
                            nc.scalar.copy(otile[:, 1024 * h : 1024 * (h + 1)], pss[h][:])
                        else:
                            nc.vector.tensor_copy(
                                otile[:, 1024 * h : 1024 * (h + 1)], pss[h][:]
                            )
                    nc.scalar.copy(otile[:, 2048:2112], csim0[:])
                    nc.scalar.copy(otile[:, 2112:2176], csim1[:])
                    nc.sync.dma_start(
                        out=sims_out[(r * NBLK + tau) * 128 : (r * NBLK + tau + 1) * 128, :],
                        in_=otile[:],
                    )

    split_excess_waits(nc)
    return nc


_PROGRAM = None


def _get_program():
    global _PROGRAM
    if _PROGRAM is None:
        _PROGRAM = build_program()
    return _PROGRAM


def kernel(z, c, negative_inds, _trace=False):
    z = np.ascontiguousarray(np.asarray(z, dtype=np.float32))
    c = np.ascontiguousarray(np.asarray(c, dtype=np.float32))
    ni = np.asarray(negative_inds)
    assert z.shape == (B, F, T) and c.shape == (B, F, T + 1)

    c_sl = np.ascontiguousarray(c[:, :, 1:])  # [B, F, T]

    nc = _get_program()
    in_maps = []
    for core in range(NCORES):
        rs = slice(core * ROWS, (core + 1) * ROWS)
        in_maps.append({"z": z[rs], "c": c_sl[rs]})

    res = run_bass_kernel_spmd(nc, in_maps, list(range(NCORES)), trace=_trace)

    # [B, T, WC] fp16: all candidate similarities (already scaled by
    # 2 / (||z_t|| ||target||), i.e. final logits)
    sims = np.concatenate(
        [res.results[i]["sims"].reshape(ROWS, T, WC) for i in range(NCORES)], axis=0
    )

    # host-side index pick (pure unshard / indexing)
    n = ni.reshape(B, T, K).astype(np.int64)  # values in [0, T-2]
    neg = np.take_along_axis(sims[:, :, :T], n, axis=2)  # [B, T, K]
    tmod = (np.arange(T) % 128)[None, :, None]
    pos = np.take_along_axis(sims[:, :, T:], tmod, axis=2)  # [B, T, 1]
    logits = np.concatenate([pos, neg], axis=2).astype(np.float32)
    out = logits.reshape(B * T, K + 1)
    if _trace:
        return out, res
    return out


if __name__ == "__main__":
    rng = np.random.default_rng(0)
    z = rng.standard_normal((B, F, T), dtype=np.float32)
    c = rng.standard_normal((B, F, T + 1), dtype=np.float32)
    ni = rng.integers(0, T - 1, size=(B, T * K)).astype(np.int64)
    out = kernel(z=z, c=c, negative_inds=ni)
    print("out", out.shape, out.dtype, np.isfinite(out).all())


# revision 13
# speedup vs baseline: 1.4418x; 1.0128x over previous
"""BENDR contrastive-loss kernel for Trainium2 (8 NeuronCores).

Reference computation (see problem): for each (b, t):
  logits[b*T+t, 0]   = cos(z[b,:,t], c[b,:,t+1]) / TEMP
  logits[b*T+t, 1+k] = cos(z[b,:,t], z[b,:,n(b,t,k)]) / TEMP
with n(b,t,k) = negative_inds[b, t*K+k] (row-local), TEMP=0.5.

Strategy: data-parallel over batch (2 rows per core). On device, all the
arithmetic runs on the TensorEngine as block similarity matrices:
  - rn_z[t] = 1/||z[:,t]||, rc[t] = 1/||c[:,t+1]|| via squared tiles +
    ones-matmul partition reduction, reciprocal (DVE) + sqrt (ACT).
  - zs[:,t] = z[:,t] * rn_z[t] * sqrt(2);  cs[:,t] = c[:,t+1] * rc[t] * sqrt(2)
    (folds both cosine denominators and the 1/TEMP=2 factor).
  - per 128-wide t-block: sims = zs_blockT @ [zs_all | cs_block]  ->
    [128, 2048+128] fp32 PSUM, stored to DRAM as fp16.
Every output logit is exactly one entry of sims: the negative (t,k) is
sims[t, n(t,k)] and the positive is sims[t, 2048+t%128].  The host does the
final index-pick (pure indexing / unshard) and returns [B*T, K+1] float32.

The gather could not be done on-device at speed: GPSIMD indirect_copy
measures ~29us per 1024 indices (~2.4ms total here), ap_gather does not
compile on this toolchain, and indirect DMA gathers measured ~62ns/row with
8 SW queues.  Computing the full similarity block on the PE (128x128 MACs
per cycle) and shipping it out in fp16 is ~50x cheaper than any of those.
"""

import sys

for _p in ("/opt/trn_rl_repo",):
    if _p not in sys.path:
        sys.path.append(_p)

import numpy as np

import concourse.bass as bass
import concourse.mybir as mybir
from concourse import tile as _tile
from concourse.tile import TileContext
from concourse.bass_utils import run_bass_kernel_spmd

dt = mybir.dt



B, F, T, K = 16, 256, 2048, 20
NCORES = 8
ROWS = B // NCORES          # batch rows per core
NBLK = T // 128             # t-blocks per batch row
WC = T + 128                # sims columns: 2048 z-sims + 128 c-diag block
FCH = F // 128              # f chunks (partition dim)

# ---------------------------------------------------------------------------
# Walrus in this container rejects instructions that carry more than one
# semaphore wait ("Too many sync wait commands").  Two shims fix that: the
# tile tail drain gets its waits on single-wait NOPs, and a post-pass splits
# any remaining multi-wait instruction.
# ---------------------------------------------------------------------------


def _patched_drain_and_barrier(self, tick_clock, wait_clock):
    nop0 = self.nc.sync.nop(nofuse=True, hint="tail_wait")
    wait_clock.add_sem_waits(
        nop0.ins, _tile.ScopedClock({None: tick_clock.global_clock})
    )
    si = nop0.ins.sync_info
    if si is not None and len(si.on_wait) > 1:
        waits = list(si.on_wait)
        nop0.ins.sync_info = mybir.SyncInfo(
            on_wait=waits[:1], on_update=list(si.on_update)
        )
        for w in waits[1:]:
            nopi = self.nc.sync.nop(nofuse=True, hint="tail_wait")
            nopi.ins.sync_info = mybir.SyncInfo(on_wait=[w], on_update=[])
    self.nc.sync.drain()
    self.nc.all_engine_barrier()
    assert self.sems is not None
    popped = self.nc._tile_sem_poison_stack.pop()
    assert popped is self._sem_poison
    self.nc.clear_and_free_semaphores(list(self.sems.allocated().values()))
    self.nc.all_engine_barrier()


_tile.TileContext._drain_and_barrier = _patched_drain_and_barrier

_wnop_counter = [0]


def split_excess_waits(nc, cap=1):
    for f in nc.m.functions:
        for bb in f.blocks:
            insts = bb.instructions
            out = []
            changed = False
            for inst in list(insts):
                si = getattr(inst, "sync_info", None)
                waits = list(si.on_wait) if si is not None else []
                if len(waits) > cap:
                    keep = waits[-cap:]
                    for w in waits[: len(waits) - cap]:
                        _wnop_counter[0] += 1
                        nop = mybir.InstNoOp(
                            name=f"wnop-{_wnop_counter[0]}", ins=[], outs=[]
                        )
                        nop.engine = inst.engine
                        nop.sync_info = mybir.SyncInfo(on_wait=[w], on_update=[])
                        out.append(nop)
                    inst.sync_info = mybir.SyncInfo(
                        on_wait=keep, on_update=list(si.on_update)
                    )
                    changed = True
                out.append(inst)
            if changed:
                insts[:] = out


def dedup_ldweights(nc):
    """The tile lowering emits an explicit InstLdweights before every
    InstMatmult.  Consecutive matmuls that share the stationary operand
    (same AP + tile position) don't need the reload -- the PE keeps its
    weights.  Convert redundant loads into NoOps (keeping their sync info)."""
    n = 0
    for f in nc.m.functions:
        for bb in f.blocks:
            insts = bb.instructions
            last_key = None
            out = []
            changed = False
            for inst in list(insts):
                tn = type(inst).__name__
                if tn == "InstLdweights":
                    key = (
                        str(inst.ins[0]),
                        tuple(inst.tile_position or ()),
                        tuple(inst.tile_size or ()),
                        bool(inst.is_transpose),
                    )
                    if key == last_key:
                        nop = mybir.InstNoOp(name=f"ldwnop-{n}", ins=[], outs=[])
                        n += 1
                        nop.engine = inst.engine
                        si = inst.sync_info
                        if si is not None:
                            nop.sync_info = mybir.SyncInfo(
                                on_wait=list(si.on_wait), on_update=list(si.on_update)
                            )
                        out.append(nop)
                        changed = True
                        continue
                    last_key = key
                elif tn == "InstMatmult":
                    if inst.is_transpose:
                        last_key = None
                out.append(inst)
            if changed:
                insts[:] = out
    return n


# ---------------------------------------------------------------------------
# Device program
# ---------------------------------------------------------------------------


def build_program():
    nc = bass.Bass("TRN2", num_devices=NCORES)
    z_in = nc.dram_tensor("z", [ROWS, F, T], dt.float32, kind="ExternalInput")
    c_in = nc.dram_tensor("c", [ROWS, F, T], dt.float32, kind="ExternalInput")
    sims_out = nc.dram_tensor(
        "sims", [ROWS * NBLK * 128, WC], dt.float16, kind="ExternalOutput"
    )

    with TileContext(nc) as tc:
        with (
            tc.tile_pool(name="io", bufs=1) as io_pool,
            tc.tile_pool(name="work", bufs=1) as work,
            tc.tile_pool(name="scaled", bufs=2) as scaled,
            tc.tile_pool(name="outp", bufs=3) as outp,
            tc.tile_pool(name="gram_ps", bufs=3, space="PSUM") as gram_ps,
            tc.tile_pool(name="stat_ps", bufs=1, space="PSUM") as stat_ps,
        ):
            ones16 = io_pool.tile([128, 1], dt.bfloat16, name="ones16")
            nc.vector.memset(ones16[:], 1.0)

            scaled_ops = []

            for r in range(ROWS):
                # ---- load + convert ----
                zf = []
                cf = []
                z16 = []
                c16 = []
                for j in range(FCH):
                    zfj = io_pool.tile([128, T], dt.float32, name=f"zf{j}", tag=f"zf{j}")
                    nc.sync.dma_start(out=zfj[:], in_=z_in[r, 128 * j : 128 * (j + 1), :])
                    zf.append(zfj)
                    cfj = io_pool.tile([128, T], dt.float32, name=f"cf{j}", tag=f"cf{j}")
                    nc.sync.dma_start(out=cfj[:], in_=c_in[r, 128 * j : 128 * (j + 1), :])
                    cf.append(cfj)
                    z16j = work.tile([128, T], dt.bfloat16, name=f"z16{j}", tag=f"z16{j}")
                    nc.scalar.copy(z16j[:], zfj[:])
                    z16.append(z16j)
                    c16j = work.tile([128, T], dt.bfloat16, name=f"c16{j}", tag=f"c16{j}")
                    nc.scalar.copy(c16j[:], cfj[:])
                    c16.append(c16j)

                # ---- squared tiles (bf16, in place) + ones-matmul reduce ----
                for j in range(FCH):
                    nc.vector.tensor_tensor(
                        out=z16[j][:], in0=z16[j][:], in1=z16[j][:], op=mybir.AluOpType.mult
                    )
                    nc.vector.tensor_tensor(
                        out=c16[j][:], in0=c16[j][:], in1=c16[j][:], op=mybir.AluOpType.mult
                    )
                # rn = sqrt(2 / normsq): ones-matmuls land the 4 column chunks
                # on partitions {0,32,64,96} so reciprocal runs on 4 DVE lanes
                nz_ps = stat_ps.tile([128, 512], dt.float32, name="nz_ps", tag="aux")
                ncc_ps = stat_ps.tile([128, 512], dt.float32, name="ncc_ps", tag="aux2")
                for cchunk in range(T // 512):
                    sl = slice(512 * cchunk, 512 * (cchunk + 1))
                    bp = 32 * cchunk
                    tp = (0, bp)
                    for j in range(FCH):
                        nc.tensor.matmul(
                            nz_ps[bp : bp + 1, :], ones16[:], z16[j][:, sl],
                            start=(j == 0), stop=(j == FCH - 1), tile_position=tp,
                        )
                        nc.tensor.matmul(
                            ncc_ps[bp : bp + 1, :], ones16[:], c16[j][:, sl],
                            start=(j == 0), stop=(j == FCH - 1), tile_position=tp,
                        )
                rz32 = work.tile([128, 512], dt.float32, name="rz32", tag="rz32")
                rc32 = work.tile([128, 512], dt.float32, name="rc32", tag="rc32")
                nc.vector.reciprocal(rz32[:], nz_ps[:])
                nc.vector.reciprocal(rc32[:], ncc_ps[:])
                rnz = work.tile([128, T], dt.float32, name="rnz", tag="rnz")
                rnc = work.tile([128, T], dt.float32, name="rnc", tag="rnc")
                for cchunk in range(T // 512):
                    sl = slice(512 * cchunk, 512 * (cchunk + 1))
                    bp = 32 * cchunk
                    nc.scalar.activation(
                        rnz[0:1, sl], rz32[bp : bp + 1, :],
                        mybir.ActivationFunctionType.Sqrt, scale=2.0,
                    )
                    nc.scalar.activation(
                        rnc[0:1, sl], rc32[bp : bp + 1, :],
                        mybir.ActivationFunctionType.Sqrt, scale=2.0,
                    )
                # replicate row 0 -> all 128 partitions (log doubling)
                kk = 1
                while kk < 128:
                    nc.sync.dma_start(out=rnz[kk : 2 * kk, :], in_=rnz[0:kk, :])
                    nc.sync.dma_start(out=rnc[kk : 2 * kk, :], in_=rnc[0:kk, :])
                    kk *= 2

                # ---- scaled operands (scale in place into zf/cf) ----
                zs16 = []
                cs16 = []
                for j in range(FCH):
                    nc.vector.tensor_tensor(
                        out=zf[j][:], in0=zf[j][:], in1=rnz[:], op=mybir.AluOpType.mult
                    )
                    zs16j = scaled.tile([128, T], dt.bfloat16, name=f"zs16{j}", tag=f"zs16{j}")
                    nc.scalar.copy(zs16j[:], zf[j][:])
                    zs16.append(zs16j)
                    nc.vector.tensor_tensor(
                        out=cf[j][:], in0=cf[j][:], in1=rnc[:], op=mybir.AluOpType.mult
                    )
                    cs16j = scaled.tile([128, T], dt.bfloat16, name=f"cs16{j}", tag=f"cs16{j}")
                    nc.scalar.copy(cs16j[:], cf[j][:])
                    cs16.append(cs16j)
                scaled_ops.append((zs16, cs16))

            for r in range(ROWS):
                zs16, cs16 = scaled_ops[r]
                # ---- per t-block similarity matrices ----
                # j-outer so the stationary operand is loaded once per f-chunk
                for tau in range(NBLK):
                    t0 = 128 * tau
                    otile = outp.tile([128, WC], dt.float16, name="otile", tag="otile")
                    ps0 = gram_ps.tile([128, 1024], dt.float32, name="ps0", tag="ps_z")
                    ps1 = gram_ps.tile([128, 1024], dt.float32, name="ps1", tag="ps_z")
                    csim0 = stat_ps.tile([128, 64], dt.float32, name="csim0", tag="aux")
                    csim1 = stat_ps.tile([128, 64], dt.float32, name="csim1", tag="aux2")
                    csims = (csim0, csim1)
                    pss = (ps0, ps1)
                    for j in range(FCH):
                        lhsT = zs16[j][:, t0 : t0 + 128]
                        st = j == 0
                        sp = j == FCH - 1
                        for h in range(2):
                            ps = pss[h]
                            nc.tensor.matmul(
                                ps[:, 0:512], lhsT,
                                zs16[j][:, 1024 * h : 1024 * h + 512],
                                start=st, stop=sp,
                            )
                            nc.tensor.matmul(
                                ps[:, 512:1024], lhsT,
                                zs16[j][:, 1024 * h + 512 : 1024 * h + 1024],
                                start=st, stop=sp,
                            )
                            nc.tensor.matmul(
                                csims[h][:], lhsT,
                                cs16[j][:, t0 + 64 * h : t0 + 64 * h + 64],
                                start=st, stop=sp,
                            )
                    for h in range(2):
                        # alternate PSUM->SBUF copies between ACT and DVE
                        if (tau + h) % 2 == 0:
                            nc.scalar.copy(otile[:, 1024 * h : 1024 * (h + 1)], pss[h][:])
                        else:
                            nc.vector.tensor_copy(
                                otile[:, 1024 * h : 1024 * (h + 1)], pss[h][:]
                            )
                    nc.scalar.copy(otile[:, 2048:2112], csim0[:])
                    nc.scalar.copy(otile[:, 2112:2176], csim1[:])
                    nc.sync.dma_start(
                        out=sims_out[(r * NBLK + tau) * 128 : (r * NBLK + tau + 1) * 128, :],
                        in_=otile[:],
                    )

    dedup_ldweights(nc)
    split_excess_waits(nc)
    return nc


_PROGRAM = None


def _get_program():
    global _PROGRAM
    if _PROGRAM is None:
        _PROGRAM = build_program()
    return _PROGRAM


def kernel(z, c, negative_inds, _trace=False):
    z = np.ascontiguousarray(np.asarray(z, dtype=np.float32))
    c = np.ascontiguousarray(np.asarray(c, dtype=np.float32))
    ni = np.asarray(negative_inds)
    assert z.shape == (B, F, T) and c.shape == (B, F, T + 1)

    c_sl = np.ascontiguousarray(c[:, :, 1:])  # [B, F, T]

    nc = _get_program()
    in_maps = []
    for core in range(NCORES):
        rs = slice(core * ROWS, (core + 1) * ROWS)
        in_maps.append({"z": z[rs], "c": c_sl[rs]})

    res = run_bass_kernel_spmd(nc, in_maps, list(range(NCORES)), trace=_trace)

    # [B, T, WC] fp16: all candidate similarities (already scaled by
    # 2 / (||z_t|| ||target||), i.e. final logits)
    sims = np.concatenate(
        [res.results[i]["sims"].reshape(ROWS, T, WC) for i in range(NCORES)], axis=0
    )

    # host-side index pick (pure unshard / indexing)
    n = ni.reshape(B, T, K).astype(np.int64)  # values in [0, T-2]
    neg = np.take_along_axis(sims[:, :, :T], n, axis=2)  # [B, T, K]
    tmod = (np.arange(T) % 128)[None, :, None]
    pos = np.take_along_axis(sims[:, :, T:], tmod, axis=2)  # [B, T, 1]
    logits = np.concatenate([pos, neg], axis=2).astype(np.float32)
    out = logits.reshape(B * T, K + 1)
    if _trace:
        return out, res
    return out


if __name__ == "__main__":
    rng = np.random.default_rng(0)
    z = rng.standard_normal((B, F, T), dtype=np.float32)
    c = rng.standard_normal((B, F, T + 1), dtype=np.float32)
    ni = rng.integers(0, T - 1, size=(B, T * K)).astype(np.int64)
    out = kernel(z=z, c=c, negative_inds=ni)
    print("out", out.shape, out.dtype, np.isfinite(out).all())


# revision 14
# speedup vs baseline: 1.5734x; 1.0913x over previous
"""BENDR contrastive-loss kernel for Trainium2 (8 NeuronCores).

Reference computation (see problem): for each (b, t):
  logits[b*T+t, 0]   = cos(z[b,:,t], c[b,:,t+1]) / TEMP
  logits[b*T+t, 1+k] = cos(z[b,:,t], z[b,:,n(b,t,k)]) / TEMP
with n(b,t,k) = negative_inds[b, t*K+k] (row-local), TEMP=0.5.

Strategy: data-parallel over batch (2 rows per core). On device, all the
arithmetic runs on the TensorEngine as block similarity matrices:
  - rn_z[t] = 1/||z[:,t]||, rc[t] = 1/||c[:,t+1]|| via squared tiles +
    ones-matmul partition reduction, reciprocal (DVE) + sqrt (ACT).
  - zs[:,t] = z[:,t] * rn_z[t] * sqrt(2);  cs[:,t] = c[:,t+1] * rc[t] * sqrt(2)
    (folds both cosine denominators and the 1/TEMP=2 factor).
  - per 128-wide t-block: sims = zs_blockT @ [zs_all | cs_block]  ->
    [128, 2048+128] fp32 PSUM, stored to DRAM as fp16.
Every output logit is exactly one entry of sims: the negative (t,k) is
sims[t, n(t,k)] and the positive is sims[t, 2048+t%128].  The host does the
final index-pick (pure indexing / unshard) and returns [B*T, K+1] float32.

The gather could not be done on-device at speed: GPSIMD indirect_copy
measures ~29us per 1024 indices (~2.4ms total here), ap_gather does not
compile on this toolchain, and indirect DMA gathers measured ~62ns/row with
8 SW queues.  Computing the full similarity block on the PE (128x128 MACs
per cycle) and shipping it out in fp16 is ~50x cheaper than any of those.
"""

import sys

for _p in ("/opt/trn_rl_repo",):
    if _p not in sys.path:
        sys.path.append(_p)

import numpy as np

import concourse.bass as bass
import concourse.mybir as mybir
from concourse import tile as _tile
from concourse.tile import TileContext
from concourse.bass_utils import run_bass_kernel_spmd

dt = mybir.dt



B, F, T, K = 16, 256, 2048, 20
NCORES = 8
ROWS = B // NCORES          # batch rows per core
NBLK = T // 128             # t-blocks per batch row
WC = T + 128                # sims columns: 2048 z-sims + 128 c-diag block
FCH = F // 128              # f chunks (partition dim)

# ---------------------------------------------------------------------------
# Walrus in this container rejects instructions that carry more than one
# semaphore wait ("Too many sync wait commands").  Two shims fix that: the
# tile tail drain gets its waits on single-wait NOPs, and a post-pass splits
# any remaining multi-wait instruction.
# ---------------------------------------------------------------------------


def _patched_drain_and_barrier(self, tick_clock, wait_clock):
    nop0 = self.nc.sync.nop(nofuse=True, hint="tail_wait")
    wait_clock.add_sem_waits(
        nop0.ins, _tile.ScopedClock({None: tick_clock.global_clock})
    )
    si = nop0.ins.sync_info
    if si is not None and len(si.on_wait) > 1:
        waits = list(si.on_wait)
        nop0.ins.sync_info = mybir.SyncInfo(
            on_wait=waits[:1], on_update=list(si.on_update)
        )
        for w in waits[1:]:
            nopi = self.nc.sync.nop(nofuse=True, hint="tail_wait")
            nopi.ins.sync_info = mybir.SyncInfo(on_wait=[w], on_update=[])
    self.nc.sync.drain()
    self.nc.all_engine_barrier()
    assert self.sems is not None
    popped = self.nc._tile_sem_poison_stack.pop()
    assert popped is self._sem_poison
    self.nc.clear_and_free_semaphores(list(self.sems.allocated().values()))
    self.nc.all_engine_barrier()


_tile.TileContext._drain_and_barrier = _patched_drain_and_barrier

_wnop_counter = [0]


def split_excess_waits(nc, cap=1):
    for f in nc.m.functions:
        for bb in f.blocks:
            insts = bb.instructions
            out = []
            changed = False
            for inst in list(insts):
                si = getattr(inst, "sync_info", None)
                waits = list(si.on_wait) if si is not None else []
                if len(waits) > cap:
                    keep = waits[-cap:]
                    for w in waits[: len(waits) - cap]:
                        _wnop_counter[0] += 1
                        nop = mybir.InstNoOp(
                            name=f"wnop-{_wnop_counter[0]}", ins=[], outs=[]
                        )
                        nop.engine = inst.engine
                        nop.sync_info = mybir.SyncInfo(on_wait=[w], on_update=[])
                        out.append(nop)
                    inst.sync_info = mybir.SyncInfo(
                        on_wait=keep, on_update=list(si.on_update)
                    )
                    changed = True
                out.append(inst)
            if changed:
                insts[:] = out


def dedup_ldweights(nc):
    """The tile lowering emits an explicit InstLdweights before every
    InstMatmult.  Consecutive matmuls that share the stationary operand
    (same AP + tile position) don't need the reload -- the PE keeps its
    weights.  Convert redundant loads into NoOps (keeping their sync info)."""
    n = 0
    for f in nc.m.functions:
        for bb in f.blocks:
            insts = bb.instructions
            last_key = None
            out = []
            changed = False
            for inst in list(insts):
                tn = type(inst).__name__
                if tn == "InstLdweights":
                    key = (
                        str(inst.ins[0]),
                        tuple(inst.tile_position or ()),
                        tuple(inst.tile_size or ()),
                        bool(inst.is_transpose),
                    )
                    if key == last_key:
                        nop = mybir.InstNoOp(name=f"ldwnop-{n}", ins=[], outs=[])
                        n += 1
                        nop.engine = inst.engine
                        si = inst.sync_info
                        if si is not None:
                            nop.sync_info = mybir.SyncInfo(
                                on_wait=list(si.on_wait), on_update=list(si.on_update)
                            )
                        out.append(nop)
                        changed = True
                        continue
                    last_key = key
                elif tn == "InstMatmult":
                    if inst.is_transpose:
                        last_key = None
                out.append(inst)
            if changed:
                insts[:] = out
    return n


# ---------------------------------------------------------------------------
# Device program
# ---------------------------------------------------------------------------


def build_program():
    nc = bass.Bass("TRN2", num_devices=NCORES)
    z_in = nc.dram_tensor("z", [ROWS, F, T], dt.float32, kind="ExternalInput")
    c_in = nc.dram_tensor("c", [ROWS, F, T], dt.float32, kind="ExternalInput")
    sims_out = nc.dram_tensor(
        "sims", [ROWS * NBLK * 128, WC], dt.float16, kind="ExternalOutput"
    )

    with TileContext(nc) as tc:
        with (
            tc.tile_pool(name="io", bufs=2) as io_pool,
            tc.tile_pool(name="work", bufs=1) as work,
            tc.tile_pool(name="scaled", bufs=2) as scaled,
            tc.tile_pool(name="outp", bufs=3) as outp,
            tc.tile_pool(name="gram_ps", bufs=3, space="PSUM") as gram_ps,
            tc.tile_pool(name="stat_ps", bufs=1, space="PSUM") as stat_ps,
        ):
            ones16 = io_pool.tile([128, 1], dt.bfloat16, name="ones16")
            nc.vector.memset(ones16[:], 1.0)

            scaled_ops = []

            for r in range(ROWS):
                nc.enter_named_scope(f"stats_r{r}", False)
                # ---- load + convert ----
                zf = []
                cf = []
                z16 = []
                c16 = []
                for j in range(FCH):
                    zfj = io_pool.tile([128, T], dt.float32, name=f"zf{j}", tag=f"zf{j}")
                    nc.sync.dma_start(out=zfj[:], in_=z_in[r, 128 * j : 128 * (j + 1), :])
                    zf.append(zfj)
                    cfj = io_pool.tile([128, T], dt.float32, name=f"cf{j}", tag=f"cf{j}")
                    nc.sync.dma_start(out=cfj[:], in_=c_in[r, 128 * j : 128 * (j + 1), :])
                    cf.append(cfj)
                    z16j = work.tile([128, T], dt.bfloat16, name=f"z16{j}", tag=f"z16{j}")
                    nc.scalar.copy(z16j[:], zfj[:])
                    z16.append(z16j)
                    c16j = work.tile([128, T], dt.bfloat16, name=f"c16{j}", tag=f"c16{j}")
                    nc.scalar.copy(c16j[:], cfj[:])
                    c16.append(c16j)

                # ---- squared tiles (bf16, in place) + ones-matmul reduce ----
                for j in range(FCH):
                    nc.vector.tensor_tensor(
                        out=z16[j][:], in0=z16[j][:], in1=z16[j][:], op=mybir.AluOpType.mult
                    )
                    nc.vector.tensor_tensor(
                        out=c16[j][:], in0=c16[j][:], in1=c16[j][:], op=mybir.AluOpType.mult
                    )
                # rn = sqrt(2 / normsq): ones-matmuls land the 4 column chunks
                # on partitions {0,32,64,96} so reciprocal runs on 4 DVE lanes
                nz_ps = stat_ps.tile([128, 512], dt.float32, name="nz_ps", tag="aux")
                ncc_ps = stat_ps.tile([128, 512], dt.float32, name="ncc_ps", tag="aux2")
                for cchunk in range(T // 512):
                    sl = slice(512 * cchunk, 512 * (cchunk + 1))
                    bp = 32 * cchunk
                    tp = (0, bp)
                    for j in range(FCH):
                        nc.tensor.matmul(
                            nz_ps[bp : bp + 1, :], ones16[:], z16[j][:, sl],
                            start=(j == 0), stop=(j == FCH - 1), tile_position=tp,
                        )
                        nc.tensor.matmul(
                            ncc_ps[bp : bp + 1, :], ones16[:], c16[j][:, sl],
                            start=(j == 0), stop=(j == FCH - 1), tile_position=tp,
                        )
                rz32 = work.tile([128, 512], dt.float32, name="rz32", tag="rz32")
                rc32 = work.tile([128, 512], dt.float32, name="rc32", tag="rc32")
                nc.vector.reciprocal(rz32[:], nz_ps[:])
                nc.vector.reciprocal(rc32[:], ncc_ps[:])
                rnz = work.tile([128, T], dt.float32, name="rnz", tag="rnz")
                rnc = work.tile([128, T], dt.float32, name="rnc", tag="rnc")
                for cchunk in range(T // 512):
                    sl = slice(512 * cchunk, 512 * (cchunk + 1))
                    bp = 32 * cchunk
                    nc.scalar.activation(
                        rnz[0:1, sl], rz32[bp : bp + 1, :],
                        mybir.ActivationFunctionType.Sqrt, scale=2.0,
                    )
                    nc.scalar.activation(
                        rnc[0:1, sl], rc32[bp : bp + 1, :],
                        mybir.ActivationFunctionType.Sqrt, scale=2.0,
                    )
                # replicate row 0 -> all 128 partitions (log doubling)
                kk = 1
                while kk < 128:
                    nc.sync.dma_start(out=rnz[kk : 2 * kk, :], in_=rnz[0:kk, :])
                    nc.sync.dma_start(out=rnc[kk : 2 * kk, :], in_=rnc[0:kk, :])
                    kk *= 2

                # ---- scaled operands (scale in place into zf/cf) ----
                zs16 = []
                cs16 = []
                for j in range(FCH):
                    nc.vector.tensor_tensor(
                        out=zf[j][:], in0=zf[j][:], in1=rnz[:], op=mybir.AluOpType.mult
                    )
                    zs16j = scaled.tile([128, T], dt.bfloat16, name=f"zs16{j}", tag=f"zs16{j}")
                    nc.scalar.copy(zs16j[:], zf[j][:])
                    zs16.append(zs16j)
                    nc.vector.tensor_tensor(
                        out=cf[j][:], in0=cf[j][:], in1=rnc[:], op=mybir.AluOpType.mult
                    )
                    cs16j = scaled.tile([128, T], dt.bfloat16, name=f"cs16{j}", tag=f"cs16{j}")
                    nc.scalar.copy(cs16j[:], cf[j][:])
                    cs16.append(cs16j)
                scaled_ops.append((zs16, cs16))
                nc.leave_named_scope(f"stats_r{r}", None, False)

            for r in range(ROWS):
                nc.enter_named_scope(f"gram_r{r}", False)
                zs16, cs16 = scaled_ops[r]
                # ---- per t-block similarity matrices ----
                # j-outer so the stationary operand is loaded once per f-chunk
                for tau in range(NBLK):
                    t0 = 128 * tau
                    otile = outp.tile([128, WC], dt.float16, name="otile", tag="otile")
                    ps0 = gram_ps.tile([128, 1024], dt.float32, name="ps0", tag="ps_z")
                    ps1 = gram_ps.tile([128, 1024], dt.float32, name="ps1", tag="ps_z")
                    csim0 = stat_ps.tile([128, 64], dt.float32, name="csim0", tag="aux")
                    csim1 = stat_ps.tile([128, 64], dt.float32, name="csim1", tag="aux2")
                    csims = (csim0, csim1)
                    pss = (ps0, ps1)
                    for j in range(FCH):
                        lhsT = zs16[j][:, t0 : t0 + 128]
                        st = j == 0
                        sp = j == FCH - 1
                        for h in range(2):
                            ps = pss[h]
                            nc.tensor.matmul(
                                ps[:, 0:512], lhsT,
                                zs16[j][:, 1024 * h : 1024 * h + 512],
                                start=st, stop=sp,
                            )
                            nc.tensor.matmul(
                                ps[:, 512:1024], lhsT,
                                zs16[j][:, 1024 * h + 512 : 1024 * h + 1024],
                                start=st, stop=sp,
                            )
                            nc.tensor.matmul(
                                csims[h][:], lhsT,
                                cs16[j][:, t0 + 64 * h : t0 + 64 * h + 64],
                                start=st, stop=sp,
                            )
                    for h in range(2):
                        # alternate PSUM->SBUF copies between ACT and DVE
                        if (tau + h) % 2 == 0:
                            nc.scalar.copy(otile[:, 1024 * h : 1024 * (h + 1)], pss[h][:])
                        else:
                            nc.vector.tensor_copy(
                                otile[:, 1024 * h : 1024 * (h + 1)], pss[h][:]
                            )
                    nc.scalar.copy(otile[:, 2048:2112], csim0[:])
                    nc.scalar.copy(otile[:, 2112:2176], csim1[:])
                    nc.sync.dma_start(
                        out=sims_out[(r * NBLK + tau) * 128 : (r * NBLK + tau + 1) * 128, :],
                        in_=otile[:],
                    )
                if True:
                    nc.leave_named_scope(f"gram_r{r}", None, False)

    dedup_ldweights(nc)
    split_excess_waits(nc)
    return nc


_PROGRAM = None


def _get_program():
    global _PROGRAM
    if _PROGRAM is None:
        _PROGRAM = build_program()
    return _PROGRAM


def kernel(z, c, negative_inds, _trace=False):
    z = np.ascontiguousarray(np.asarray(z, dtype=np.float32))
    c = np.ascontiguousarray(np.asarray(c, dtype=np.float32))
    ni = np.asarray(negative_inds)
    assert z.shape == (B, F, T) and c.shape == (B, F, T + 1)

    c_sl = np.ascontiguousarray(c[:, :, 1:])  # [B, F, T]

    nc = _get_program()
    in_maps = []
    for core in range(NCORES):
        rs = slice(core * ROWS, (core + 1) * ROWS)
        in_maps.append({"z": z[rs], "c": c_sl[rs]})

    res = run_bass_kernel_spmd(nc, in_maps, list(range(NCORES)), trace=_trace)

    # [B, T, WC] fp16: all candidate similarities (already scaled by
    # 2 / (||z_t|| ||target||), i.e. final logits)
    sims = np.concatenate(
        [res.results[i]["sims"].reshape(ROWS, T, WC) for i in range(NCORES)], axis=0
    )

    # host-side index pick (pure unshard / indexing)
    n = ni.reshape(B, T, K).astype(np.int64)  # values in [0, T-2]
    neg = np.take_along_axis(sims[:, :, :T], n, axis=2)  # [B, T, K]
    tmod = (np.arange(T) % 128)[None, :, None]
    pos = np.take_along_axis(sims[:, :, T:], tmod, axis=2)  # [B, T, 1]
    logits = np.concatenate([pos, neg], axis=2).astype(np.float32)
    out = logits.reshape(B * T, K + 1)
    if _trace:
        return out, res
    return out


if __name__ == "__main__":
    rng = np.random.default_rng(0)
    z = rng.standard_normal((B, F, T), dtype=np.float32)
    c = rng.standard_normal((B, F, T + 1), dtype=np.float32)
    ni = rng.integers(0, T - 1, size=(B, T * K)).astype(np.int64)
    out = kernel(z=z, c=c, negative_inds=ni)
    print("out", out.shape, out.dtype, np.isfinite(out).all())
